# revision 1
# baseline (speedup 1.0000x reference)
"""Self-contained Trainium2 kernel for ReRoPE sparse attention.

Problem: x(2,1024,2048) -> attention with 16 Q heads / 8 KV heads (GQA),
RoPE within a 256-token causal band, ReRoPE (query rotated at fixed
position 256, keys unrotated) outside the band, -> out proj (2048x2048).

Sharding: 8 cores = 2 batches x 4 head groups. Each core computes 4 Q
heads / 2 KV heads of one batch plus its slice of all projections, and
produces a partial (1024,2048) output (wo row-parallel). Partials are
summed on the host (the per-batch all-reduce equivalent).

All device compute in bf16 (fp32 PSUM accumulation).
"""

import sys
import types
import numpy as np
import ml_dtypes

B, S, D = 2, 1024, 2048
NH, NKV, HD = 16, 8, 128
W = 256
HPC, KPC = 4, 2            # q heads / kv heads per core
KC = D // 128              # 16 contraction chunks
SB = S // 128              # 8 sequence blocks
SCALE = 1.0 / float(np.sqrt(HD))
BF16 = ml_dtypes.bfloat16

_NC_CACHE = {}


def _build_nc():
    import concourse.bass as bass
    import concourse.tile as tile
    from concourse import bacc, mybir
    from contextlib import ExitStack

    bf = mybir.dt.bfloat16
    f32 = mybir.dt.float32
    AF = mybir.ActivationFunctionType

    nc = bacc.Bacc()
    xt = nc.declare_dram_parameter("xt", [D, S], bf, isOutput=False)
    wq = nc.declare_dram_parameter("wq", [D, HPC * HD], bf, isOutput=False)
    wk = nc.declare_dram_parameter("wk", [D, KPC * HD], bf, isOutput=False)
    wv = nc.declare_dram_parameter("wv", [D, KPC * HD], bf, isOutput=False)
    wo = nc.declare_dram_parameter("wo", [HPC * HD, D], bf, isOutput=False)
    cost = nc.declare_dram_parameter("cost", [HD // 2, S], f32, isOutput=False)
    sint = nc.declare_dram_parameter("sint", [HD // 2, S], f32, isOutput=False)
    m0d = nc.declare_dram_parameter("m0", [128, 128], bf, isOutput=False)
    m2d = nc.declare_dram_parameter("m2", [128, 128], bf, isOutput=False)
    idd = nc.declare_dram_parameter("ident", [128, 128], bf, isOutput=False)
    out = nc.declare_dram_parameter("out", [S, D], bf, isOutput=True)

    with tile.TileContext(nc) as tc:
        with ExitStack() as ctx:
            p_x = ctx.enter_context(tc.tile_pool(name="p_x", bufs=KC))
            p_wq = ctx.enter_context(tc.tile_pool(name="p_wq", bufs=KC))
            p_wk = ctx.enter_context(tc.tile_pool(name="p_wk", bufs=KC))
            p_wv = ctx.enter_context(tc.tile_pool(name="p_wv", bufs=KC))
            p_wo = ctx.enter_context(tc.tile_pool(name="p_wo", bufs=HPC))
            p_tab = ctx.enter_context(tc.tile_pool(name="p_tab", bufs=2))
            p_cst = ctx.enter_context(tc.tile_pool(name="p_cst", bufs=3))
            p_q = ctx.enter_context(tc.tile_pool(name="p_q", bufs=2 * HPC))
            p_k = ctx.enter_context(tc.tile_pool(name="p_k", bufs=2 * KPC))
            p_v = ctx.enter_context(tc.tile_pool(name="p_v", bufs=SB))
            p_ao = ctx.enter_context(tc.tile_pool(name="p_ao", bufs=HPC))
            p_P = ctx.enter_context(tc.tile_pool(name="p_P", bufs=30))
            p_pt = ctx.enter_context(tc.tile_pool(name="p_pt", bufs=10))
            p_rt = ctx.enter_context(tc.tile_pool(name="p_rt", bufs=8))
            p_rc = ctx.enter_context(tc.tile_pool(name="p_rc", bufs=4))
            p_st = ctx.enter_context(tc.tile_pool(name="p_st", bufs=3))

            ps_proj = ctx.enter_context(
                tc.tile_pool(name="ps_proj", bufs=2, space="PSUM"))
            ps_attn = ctx.enter_context(
                tc.tile_pool(name="ps_attn", bufs=4, space="PSUM"))
            ps_out = ctx.enter_context(
                tc.tile_pool(name="ps_out", bufs=2, space="PSUM"))

            # ---- input DMAs (consumption order) ----
            xt_t, wk_t, wv_t, wq_t = [], [], [], []
            for t in range(KC):
                tx = p_x.tile([128, S], bf, tag="xt")
                nc.sync.dma_start(tx[:], xt[t * 128:(t + 1) * 128, :])
                xt_t.append(tx)
                tk = p_wk.tile([128, KPC * HD], bf, tag="wk")
                nc.sync.dma_start(tk[:], wk[t * 128:(t + 1) * 128, :])
                wk_t.append(tk)
                tw = p_wq.tile([128, HPC * HD], bf, tag="wq")
                nc.sync.dma_start(tw[:], wq[t * 128:(t + 1) * 128, :])
                wq_t.append(tw)
                tv = p_wv.tile([128, KPC * HD], bf, tag="wv")
                nc.sync.dma_start(tv[:], wv[t * 128:(t + 1) * 128, :])
                wv_t.append(tv)
            cos_t = p_tab.tile([HD // 2, S], f32, tag="tab")
            nc.sync.dma_start(cos_t[:], cost[:, :])
            sin_t = p_tab.tile([HD // 2, S], f32, tag="tab")
            nc.sync.dma_start(sin_t[:], sint[:, :])
            m0_t = p_cst.tile([128, 128], bf, tag="cst")
            nc.sync.dma_start(m0_t[:], m0d[:, :])
            m2_t = p_cst.tile([128, 128], bf, tag="cst")
            nc.sync.dma_start(m2_t[:], m2d[:, :])
            id_t = p_cst.tile([128, 128], bf, tag="cst")
            nc.sync.dma_start(id_t[:], idd[:, :])
            wo_t = []
            for hc in range(HPC):
                tw = p_wo.tile([128, D], bf, tag="wo")
                nc.sync.dma_start(tw[:], wo[hc * 128:(hc + 1) * 128, :])
                wo_t.append(tw)

            cw = cos_t[:, W:W + 1]
            sw = sin_t[:, W:W + 1]

            def rope_var(dst, ps, half):
                """Positional rope: dst = rope(ps) for seq cols of `half`."""
                sl = slice(half * 512, (half + 1) * 512)
                c = cos_t[:, sl]
                s = sin_t[:, sl]
                qr = ps[0:64, :]
                qi = ps[64:128, :]
                t1 = p_rt.tile([64, 512], f32, tag="rt")
                nc.vector.tensor_mul(t1[:], qr, c)
                t2 = p_rt.tile([64, 512], f32, tag="rt")
                nc.vector.tensor_mul(t2[:], qi, s)
                nc.vector.tensor_sub(dst[0:64, sl], t1[:], t2[:])
                t3 = p_rt.tile([64, 512], f32, tag="rt")
                nc.vector.tensor_mul(t3[:], qr, s)
                t4 = p_rt.tile([64, 512], f32, tag="rt")
                nc.vector.tensor_mul(t4[:], qi, c)
                nc.vector.tensor_add(dst[64:128, sl], t3[:], t4[:])

            def rope_fix(dst, ps, half):
                """Fixed-angle rope at position W (per-partition scalars)."""
                sl = slice(half * 512, (half + 1) * 512)
                qr = ps[0:64, :]
                qi = ps[64:128, :]
                t1 = p_rt.tile([64, 512], f32, tag="rt")
                nc.vector.tensor_scalar_mul(t1[:], qr, cw)
                t2 = p_rt.tile([64, 512], f32, tag="rt")
                nc.vector.tensor_scalar_mul(t2[:], qi, sw)
                nc.vector.tensor_sub(dst[0:64, sl], t1[:], t2[:])
                t3 = p_rt.tile([64, 512], f32, tag="rt")
                nc.vector.tensor_scalar_mul(t3[:], qr, sw)
                t4 = p_rt.tile([64, 512], f32, tag="rt")
                nc.vector.tensor_scalar_mul(t4[:], qi, cw)
                nc.vector.tensor_add(dst[64:128, sl], t3[:], t4[:])

            # ---- K projection + rope ----
            k1_t, k2_t = [], []
            for kv in range(KPC):
                d1 = p_k.tile([128, S], bf, tag="k")
                d2 = p_k.tile([128, S], bf, tag="k")
                for half in range(2):
                    ps = ps_proj.tile([128, 512], f32, tag="proj")
                    for t in range(KC):
                        nc.tensor.matmul(
                            ps[:],
                            lhsT=wk_t[t][:, kv * 128:(kv + 1) * 128],
                            rhs=xt_t[t][:, half * 512:(half + 1) * 512],
                            start=(t == 0), stop=(t == KC - 1))
                    rope_var(d1, ps, half)
                    nc.any.tensor_copy(
                        d2[:, half * 512:(half + 1) * 512], ps[:])
                k1_t.append(d1)
                k2_t.append(d2)

            # ---- Q projection + both ropes (head 0 first for overlap) ----
            q1_t = [None] * HPC
            q2_t = [None] * HPC

            def q_proj(h):
                d1 = p_q.tile([128, S], bf, tag="q")
                d2 = p_q.tile([128, S], bf, tag="q")
                for half in range(2):
                    ps = ps_proj.tile([128, 512], f32, tag="proj")
                    for t in range(KC):
                        nc.tensor.matmul(
                            ps[:],
                            lhsT=wq_t[t][:, h * 128:(h + 1) * 128],
                            rhs=xt_t[t][:, half * 512:(half + 1) * 512],
                            start=(t == 0), stop=(t == KC - 1))
                    rope_var(d1, ps, half)
                    rope_fix(d2, ps, half)
                q1_t[h] = d1
                q2_t[h] = d2

            q_proj(0)

            # ---- V projection (natural [s, hd] layout + ones columns) ----
            v_t = []
            for sb in range(SB):
                ps = ps_proj.tile([128, KPC * HD], f32, tag="proj")
                for t in range(KC):
                    nc.tensor.matmul(
                        ps[:],
                        lhsT=xt_t[t][:, sb * 128:(sb + 1) * 128],
                        rhs=wv_t[t][:],
                        start=(t == 0), stop=(t == KC - 1))
                tv = p_v.tile([128, 2 * (HD + 1)], bf, tag="v")
                nc.any.tensor_copy(tv[:, 0:HD], ps[:, 0:HD])
                nc.any.tensor_copy(tv[:, HD + 1:2 * HD + 1], ps[:, HD:2 * HD])
                nc.vector.memset(tv[:, HD:HD + 1], 1.0)
                nc.vector.memset(tv[:, 2 * HD + 1:2 * HD + 2], 1.0)
                v_t.append(tv)

            for h in range(1, HPC):
                q_proj(h)

            # ---- attention per head ----
            ao_t = []
            for h in range(HPC):
                kv = h // 2
                Pt = {}
                ao = p_ao.tile([128, S], bf, tag="ao")
                ao_t.append(ao)
                for j in range(SB):
                    # band scores (s1): queries i in [j, j+2]
                    ihi = min(j + 2, SB - 1)
                    nb = (ihi - j + 1) * 128
                    psb = ps_attn.tile([128, nb], f32, tag="attn")
                    nc.tensor.matmul(
                        psb[:],
                        lhsT=k1_t[kv][:, j * 128:(j + 1) * 128],
                        rhs=q1_t[h][:, j * 128:(ihi + 1) * 128],
                        start=True, stop=True)
                    # far scores (s2): queries i in [j+2, 7]
                    fars = []  # (i_start, width, tile)
                    if j + 2 <= SB - 1:
                        i0 = j + 2
                        nf = (SB - i0) * 128
                        off = 0
                        while off < nf:
                            wseg = min(512, nf - off)
                            psf = ps_attn.tile([128, wseg], f32, tag="attn")
                            nc.tensor.matmul(
                                psf[:],
                                lhsT=k2_t[kv][:, j * 128:(j + 1) * 128],
                                rhs=q2_t[h][:, i0 * 128 + off:
                                            i0 * 128 + off + wseg],
                                start=True, stop=True)
                            fars.append((i0 + off // 128, wseg // 128, psf))
                            off += wseg

                    def far_slice(i):
                        for (fi, fw, psf) in fars:
                            if fi <= i < fi + fw:
                                o = (i - fi) * 128
                                return psf[:, o:o + 128]
                        raise AssertionError

                    for i in range(j, SB):
                        d = i - j
                        if d == 0:
                            pt = p_pt.tile([128, 128], bf, tag="pt")
                            nc.scalar.activation(
                                pt[:], psb[:, 0:128], AF.Exp, scale=SCALE)
                            P = p_P.tile([128, 128], bf, tag="P")
                            nc.vector.tensor_mul(P[:], pt[:], m0_t[:])
                        elif d == 1:
                            P = p_P.tile([128, 128], bf, tag="P")
                            nc.scalar.activation(
                                P[:], psb[:, 128:256], AF.Exp, scale=SCALE)
                        elif d == 2:
                            p1 = p_pt.tile([128, 128], bf, tag="pt")
                            nc.scalar.activation(
                                p1[:], psb[:, 256:384], AF.Exp, scale=SCALE)
                            p2 = p_pt.tile([128, 128], bf, tag="pt")
                            nc.scalar.activation(
                                p2[:], far_slice(i), AF.Exp, scale=SCALE)
                            pa = p_pt.tile([128, 128], bf, tag="pt")
                            nc.vector.tensor_mul(pa[:], p1[:], m2_t[:])
                            pb = p_pt.tile([128, 128], bf, tag="pt")
                            nc.vector.tensor_mul(pb[:], p2[:], m0_t[:])
                            P = p_P.tile([128, 128], bf, tag="P")
                            nc.vector.tensor_add(P[:], pa[:], pb[:])
                        else:
                            P = p_P.tile([128, 128], bf, tag="P")
                            nc.scalar.activation(
                                P[:], far_slice(i), AF.Exp, scale=SCALE)
                        Pt[(i, j)] = P

                    # row i == j now complete: attn @ v, normalize, transpose
                    i = j
                    pso = ps_attn.tile([128, HD + 1], f32, tag="attn")
                    for jj in range(i + 1):
                        nc.tensor.matmul(
                            pso[:],
                            lhsT=Pt[(i, jj)][:],
                            rhs=v_t[jj][:, kv * (HD + 1):(kv + 1) * (HD + 1)],
                            start=(jj == 0), stop=(jj == i))
                        Pt[(i, jj)] = None
                    rc = p_rc.tile([128, 1], f32, tag="rc")
                    nc.vector.reciprocal(rc[:], pso[:, HD:HD + 1])
                    an = p_pt.tile([128, 128], bf, tag="pt")
                    nc.vector.tensor_scalar_mul(an[:], pso[:, 0:HD], rc[:])
                    pst = ps_attn.tile([128, 128], bf, tag="attn")
                    nc.tensor.transpose(pst[:], an[:], id_t[:])
                    nc.any.tensor_copy(ao[:, i * 128:(i + 1) * 128], pst[:])

            # ---- output projection (row-parallel wo; partial output) ----
            for sb in range(SB):
                for cg in range(4):
                    ps = ps_out.tile([128, 512], f32, tag="out")
                    for hc in range(HPC):
                        nc.tensor.matmul(
                            ps[:],
                            lhsT=ao_t[hc][:, sb * 128:(sb + 1) * 128],
                            rhs=wo_t[hc][:, cg * 512:(cg + 1) * 512],
                            start=(hc == 0), stop=(hc == HPC - 1))
                    st = p_st.tile([128, 512], bf, tag="st")
                    nc.any.tensor_copy(st[:], ps[:])
                    nc.sync.dma_start(
                        out[sb * 128:(sb + 1) * 128,
                            cg * 512:(cg + 1) * 512], st[:])

    nc.finalize()
    return nc


def _get_nc():
    if "nc" not in _NC_CACHE:
        _NC_CACHE["nc"] = _build_nc()
    return _NC_CACHE["nc"]


def _host_inputs(x, freqs_cos, freqs_sin, wq, wk, wv, wo):
    """Build the 8 per-core input maps (host-side shard + layout prep)."""
    x = np.asarray(x, np.float32)
    wq = np.asarray(wq, np.float32)
    wk = np.asarray(wk, np.float32)
    wv = np.asarray(wv, np.float32)
    wo = np.asarray(wo, np.float32)
    perm = np.concatenate([np.arange(0, HD, 2), np.arange(1, HD, 2)])

    cos_t = np.ascontiguousarray(
        np.asarray(freqs_cos, np.float32).T)          # (64, S)
    sin_t = np.ascontiguousarray(np.asarray(freqs_sin, np.float32).T)
    ki = np.arange(128)[:, None]
    qi = np.arange(128)[None, :]
    m0 = (ki <= qi).astype(BF16)                       # causal / far-select
    m2 = (qi < ki).astype(BF16)                        # in-band select (d=2)
    ident = np.eye(128, dtype=BF16)

    wq3 = wq.reshape(D, NH, HD)
    wk3 = wk.reshape(D, NKV, HD)
    wv3 = wv.reshape(D, NKV, HD)
    wo3 = wo.reshape(NH, HD, D)

    in_maps = []
    for c in range(8):
        b, g = divmod(c, 4)
        wqc = wq3[:, 4 * g:4 * g + 4][:, :, perm].reshape(D, HPC * HD)
        wkc = wk3[:, 2 * g:2 * g + 2][:, :, perm].reshape(D, KPC * HD)
        wvc = wv3[:, 2 * g:2 * g + 2].reshape(D, KPC * HD)
        woc = wo3[4 * g:4 * g + 4].reshape(HPC * HD, D)
        in_maps.append({
            "xt": np.ascontiguousarray(x[b].T).astype(BF16),
            "wq": np.ascontiguousarray(wqc).astype(BF16),
            "wk": np.ascontiguousarray(wkc).astype(BF16),
            "wv": np.ascontiguousarray(wvc).astype(BF16),
            "wo": np.ascontiguousarray(woc).astype(BF16),
            "cost": cos_t, "sint": sin_t,
            "m0": m0, "m2": m2, "ident": ident,
        })
    return in_maps


def _run(nc, in_maps, **kw):
    from concourse.bass_utils import run_bass_kernel_spmd
    return run_bass_kernel_spmd(nc, in_maps, core_ids=list(range(8)), **kw)


def kernel(x, freqs_cos, freqs_sin, wq, wk, wv, wo):
    nc = _get_nc()
    in_maps = _host_inputs(x, freqs_cos, freqs_sin, wq, wk, wv, wo)
    res = _run(nc, in_maps)
    parts = [np.asarray(res.results[c]["out"], np.float32) for c in range(8)]
    out = np.stack([sum(parts[0:4]), sum(parts[4:8])])
    return out.astype(np.float32)
